# revision 51
# baseline (speedup 1.0000x reference)
"""Trainium2 Bass kernel for a 2-layer Longformer-style encoder.

Model: B=2, S=2048, F=438, H=768, NH=12, HD=64, one-sided window w=32, L=2.

Sharding: 8 cores, data-parallel over (batch, sequence-quarter). Each core
computes 512 output tokens from a 640-token local window (64-token halo on
each side covers the 2-layer receptive field), so no collectives are needed.

Device algorithm per core (uniform SPMD, 640 local tokens, all bf16 GEMMs):
  - x0 = srcT.T @ W_embT + pos_emb                          [token-major bf16]
  - per layer:
      xT    = PE-transpose(x) bf16, padded 32/96 cols       [feature-major]
      qT    = W_qT.T @ xT (q-scale folded into W on host)   [feature-major]
      kTp   = W_kT.T @ xT at free-offset 32 (704-wide pad)  [feature-major]
      V_ev  = shifted-window GEMM, window m = tokens [128m-32, 128m+96)
      V_od  = SBUF->SBUF DMA shift of V_ev (+64 tokens)
      attention in 20 (64-query-block, 6-head) units, software-pipelined:
        S[n,hq]  = per-head matmuls into one [128,384] PSUM tile
        S       += band+boundary mask (PE identity-matmul accumulate)
        P        = exp(S)                  one Act op, bf16 SBUF
        cd[:,0:192]  = ctx   (V.T @ P, head-pair packed)
        cd[:,192:384]= den   (ones.T @ P, pair-replicated rows)
        ctxT    += cd_ctx * recip(cd_den)  [feature-major bf16]
      fc: F = ctxT.T @ W_fcT + residual (DVE stt, accum -> sum(x))
      LN1 on Act: Square-accum for sum(x^2); x = Copy(F*rstd - m*rstd)
      H1T = relu(W_1T.T @ x1T)                              [feature-major]
      F2 = H1T.T @ W_2T + x1; LN2 -> x2 (-> xT or DMA out on last layer)
  - out = x2[64:576] f32
"""

import numpy as np
import ml_dtypes

B, S, F_DIM, H, NH, HD, W_ONE, L = 2, 2048, 438, 768, 12, 64, 32, 2
NCORES = 8
CHUNK = 512          # output tokens per core
HALO = 64            # per side
T_LOC = CHUNK + 2 * HALO   # 640 local tokens
NT = T_LOC // 128          # 5 token tiles
NB = T_LOC // 64           # 10 query blocks of 64
KPAD = T_LOC + 64          # 704 padded key width
FK = 512                   # padded embedding contraction (438 -> 512)
MASK_NEG = -50.0

bf16 = ml_dtypes.bfloat16


def _np(x):
    return np.asarray(x)


def host_prep(inputs):
    """Split full inputs into shared weight arrays + per-core arrays."""
    src_seq = _np(inputs["src_seq"]).astype(np.float32)
    src_pos = _np(inputs["src_pos"]).astype(np.int32)
    pos_table = _np(inputs["pos_table"]).astype(np.float32)

    shared = {}
    qscale = float(HD) ** -0.5

    W_emb = _np(inputs["W_emb"]).astype(np.float32)        # [H, F]
    WembT = np.zeros((FK, H), np.float32)
    WembT[:F_DIM] = W_emb.T
    shared["wembT"] = WembT.astype(bf16)

    for l in range(L):
        shared[f"wqT{l}"] = (_np(inputs["Wq"])[l].astype(np.float32).T * qscale).astype(bf16)
        shared[f"wkT{l}"] = _np(inputs["Wk"])[l].astype(np.float32).T.astype(bf16)
        shared[f"wvT{l}"] = _np(inputs["Wv"])[l].astype(np.float32).T.astype(bf16)
        shared[f"wfcT{l}"] = _np(inputs["Wfc"])[l].astype(np.float32).T.astype(bf16)
        shared[f"w1T{l}"] = _np(inputs["W1"])[l].astype(np.float32).T.astype(bf16)
        shared[f"w2T{l}"] = _np(inputs["W2"])[l].astype(np.float32).T.astype(bf16)
        shared[f"bq{l}"] = (_np(inputs["bq"])[l].astype(np.float32) * qscale)
        shared[f"bk{l}"] = _np(inputs["bk"])[l].astype(np.float32)
        shared[f"bv{l}"] = _np(inputs["bv"])[l].astype(np.float32)
        shared[f"bfc{l}"] = _np(inputs["bfc"])[l].astype(np.float32)
        shared[f"b1{l}"] = _np(inputs["b1"])[l].astype(np.float32)
        shared[f"b2{l}"] = _np(inputs["b2"])[l].astype(np.float32)
        shared[f"ln1g{l}"] = _np(inputs["ln1_g"])[l].astype(np.float32)
        shared[f"ln1b{l}"] = _np(inputs["ln1_b"])[l].astype(np.float32)
        shared[f"ln2g{l}"] = _np(inputs["ln2_g"])[l].astype(np.float32)
        shared[f"ln2b{l}"] = _np(inputs["ln2_b"])[l].astype(np.float32)

    b_emb = _np(inputs["b_emb"]).astype(np.float32)

    per_core = []
    for c in range(NCORES):
        b, q = divmod(c, NCORES // B)
        gstart = q * CHUNK - HALO
        lo, hi = max(gstart, 0), min(gstart + T_LOC, S)

        src_halo = np.zeros((T_LOC, F_DIM), np.float32)
        src_halo[lo - gstart: hi - gstart] = src_seq[b, lo:hi]
        srcT = np.zeros((FK, T_LOC), np.float32)
        srcT[:F_DIM] = src_halo.T

        pos_emb = np.zeros((T_LOC, H), np.float32)
        pos_emb[lo - gstart: hi - gstart] = pos_table[src_pos[b, lo:hi]]
        pos_emb += b_emb[None, :]

        # masks[n, blk, half, 6*64]: n-major additive mask per 64-query block.
        # Key validity only enforced where the query itself is in-range, so
        # every softmax row keeps an O(1) denominator (pad-query rows are
        # discarded on output).
        masks = np.empty((128, NB, 2, 384), np.float32)
        n = np.arange(128)[:, None]
        qq = np.arange(64)[None, :]
        for blk in range(NB):
            kl = 64 * blk - 32 + n
            kg = gstart + kl
            qg = gstart + 64 * blk + qq
            band = (n - qq >= 0) & (n - qq <= 2 * W_ONE)
            keyok = (kl >= 0) & (kl < T_LOC) & (kg >= 0) & (kg < S)
            qok = (qg >= 0) & (qg < S)
            valid = band & (keyok | ~qok)
            m = np.where(valid, 1.0, 0.0).astype(np.float32)  # [128, 64]
            for g in range(2):
                masks[:, blk, g, :] = np.tile(m, (1, 6))

        per_core.append({
            "srcT": srcT.astype(bf16),
            "pos_emb": pos_emb.astype(bf16),
            "masks": masks.astype(bf16),
        })

    shared["ident"] = np.eye(128, dtype=np.float32).astype(bf16)

    flags = {}
    for l in range(L):
        for nm in ("bq", "bk", "bv", "bfc", "b1", "b2"):
            flags[f"{nm}{l}"] = not np.allclose(shared[f"{nm}{l}"], 0.0)
        for nm in ("ln1", "ln2"):
            flags[f"{nm}{l}"] = not (
                np.allclose(shared[f"{nm}g{l}"], 1.0)
                and np.allclose(shared[f"{nm}b{l}"], 0.0)
            )
    return shared, per_core, flags


def assemble(core_outs):
    out = np.zeros((B, S, H), np.float32)
    for c in range(NCORES):
        b, q = divmod(c, NCORES // B)
        out[b, q * CHUNK:(q + 1) * CHUNK] = core_outs[c]
    return out


# ---------------------------------------------------------------------------
# Bass program
# ---------------------------------------------------------------------------

def _legalize_waits(nc):
    """This container's walrus codegen accepts only ONE sync-wait per compute
    instruction ("Too many sync wait commands"). Tile's scheduler emits
    multi-wait instructions, so split: keep the last wait on the instruction
    and carry earlier ones on same-engine NoOps inserted right before it."""
    import concourse.mybir as mybir

    act_nm, pool_nm = getattr(nc, "_carrier_names", (None, None))
    tmplA = tmplP = None
    for fn in nc.m.functions:
        for blk in fn.blocks:
            for inst in blk.instructions:
                if inst.name == act_nm:
                    tmplA = inst
                elif inst.name == pool_nm:
                    tmplP = inst

    for fn in nc.m.functions:
        for blk in fn.blocks:
            out = []
            changed = False
            for inst in blk.instructions:
                si = getattr(inst, "sync_info", None)
                waits = list(si.on_wait) if si is not None and si.on_wait else []
                if len(waits) > 1 and not isinstance(
                        inst, mybir.InstEventSemaphore):
                    for j, w in enumerate(waits[:-1]):
                        if (inst.engine == mybir.EngineType.Activation
                                and CARRIERS and tmplA is not None):
                            nop = mybir.InstActivation(
                                name=f"{inst.name}-w{j}",
                                func=mybir.ActivationFunctionType.Copy,
                                ins=list(tmplA.ins), outs=list(tmplA.outs))
                        elif (inst.engine == mybir.EngineType.Pool
                                and CARRIERS and tmplP is not None):
                            nop = mybir.InstMemset(
                                name=f"{inst.name}-w{j}", mode=tmplP.mode,
                                ins=[], outs=list(tmplP.outs),
                                constant=tmplP.constant)
                        elif inst.engine in (mybir.EngineType.Activation,
                                             mybir.EngineType.Pool):
                            nop = mybir.InstDrain(
                                name=f"{inst.name}-w{j}", ins=[], outs=[])
                        else:
                            nop = mybir.InstNoOp(
                                name=f"{inst.name}-w{j}", ins=[], outs=[])
                        nop.engine = inst.engine
                        nop.sync_info = mybir.SyncInfo(on_wait=[w], on_update=[])
                        out.append(nop)
                    inst.sync_info = mybir.SyncInfo(
                        on_wait=[waits[-1]], on_update=list(si.on_update or []))
                    changed = True
                out.append(inst)
            if changed:
                blk.instructions = out


def _act_reciprocal(nc, mybir, out, in_):
    """ACT-engine LUT reciprocal. bass raises on ActivationFunctionType.
    Reciprocal citing accuracy issues, but measured on this hardware it is
    ~1e-5 relative over [1e-6, 1e3] - plenty for softmax denominators."""
    eng = nc.scalar
    inputs = [eng.lower_ap(in_)]
    for arg in (0.0, 1.0, 0.0):
        inputs.append(mybir.ImmediateValue(dtype=mybir.dt.float32, value=arg))
    return eng.add_instruction(mybir.InstActivation(
        name=nc.get_next_instruction_name(),
        func=mybir.ActivationFunctionType.Reciprocal,
        ins=inputs, outs=[eng.lower_ap(out)]))


ATTN_SUB = 9  # 1=front only 2=+den 3=+ctx 4=+recip 5=+mult
LIMIT = 99  # bisection knob: 1=embed 2=+qk 3=+v 4=+attn 5=+fc/ln1 6=+w1 7=+w2/ln2 (x10 per layer)

def build_program(flags):
    import concourse.bass as bass
    import concourse.mybir as mybir
    import concourse.tile as tile
    import contextlib

    f32 = mybir.dt.float32
    bf = mybir.dt.bfloat16
    AF = mybir.ActivationFunctionType
    ALU = mybir.AluOpType

    nc = bass.Bass()
    FT = H // 128          # 6 feature tiles
    KTE = FK // 128        # 4 embedding contraction tiles
    INV_H = 1.0 / H

    # ---- DRAM tensors ----
    D = {}
    names = []

    def din(name, shape, dt):
        D[name] = nc.dram_tensor(name, shape, dt, kind="ExternalInput")
        names.append(name)

    din("srcT", [FK, T_LOC], bf)
    din("pos_emb", [T_LOC, H], bf)
    din("masks", [128, NB, 2, 384], bf)
    din("ident", [128, 128], bf)
    din("wembT", [FK, H], bf)
    for l in range(L):
        for nm in ("wqT", "wkT", "wvT", "wfcT", "w1T", "w2T"):
            din(f"{nm}{l}", [H, H], bf)
        for nm in ("bq", "bk", "bv", "bfc", "b1", "b2"):
            if flags[f"{nm}{l}"]:
                din(f"{nm}{l}", [H], f32)
        for nm in ("ln1", "ln2"):
            if flags[f"{nm}{l}"]:
                din(f"{nm}g{l}", [H], f32)
                din(f"{nm}b{l}", [H], f32)
    out_d = nc.dram_tensor("out", [CHUNK, H], f32, kind="ExternalOutput")

    def bcast_ap(dram, n):
        return bass.AP(tensor=dram.tensor, offset=dram.offset, ap=[[0, 128], [1, n]])

    with tile.TileContext(nc) as tc:
        with contextlib.ExitStack() as ctx:
            consts = ctx.enter_context(tc.tile_pool(name="consts", bufs=1))
            acts = ctx.enter_context(tc.tile_pool(name="acts", bufs=1))
            fpool = ctx.enter_context(tc.tile_pool(name="fp", bufs=3))
            ppool = ctx.enter_context(tc.tile_pool(name="pp", bufs=6))
            rpool = ctx.enter_context(tc.tile_pool(name="rp", bufs=4))
            lpool = ctx.enter_context(tc.tile_pool(name="lp", bufs=3))
            psg = ctx.enter_context(tc.tile_pool(name="psg", bufs=2, space="PSUM"))
            pstr = ctx.enter_context(tc.tile_pool(name="pstr", bufs=1, space="PSUM"))
            pss = ctx.enter_context(tc.tile_pool(name="pss", bufs=2, space="PSUM"))
            psc = ctx.enter_context(tc.tile_pool(name="psc", bufs=3, space="PSUM"))

            # ---- constants / inputs to SBUF ----
            ident_sb = consts.tile([128, 128], bf)
            ones64 = consts.tile([128, 64], bf)
            nc.vector.memset(ones64, 1.0)
            eps_sb = consts.tile([128, 1], f32)
            nc.vector.memset(eps_sb, 1e-5)
            # 1-element template ops: legalize clones these as wait-carriers
            # on Act/Pool (a Drain there would block the whole engine)
            dcar = consts.tile([1, 1], f32, name="dcar")
            _ca = nc.scalar.activation(dcar[0:1, 0:1], eps_sb[0:1, 0:1], AF.Copy)
            _cp = nc.gpsimd.memset(dcar[0:1, 0:1], 0.0)
            nc._carrier_names = (_ca.ins.name, _cp.ins.name)

            srcT_a = consts.tile([128, 2, T_LOC], bf)
            srcT_b = consts.tile([128, 2, T_LOC], bf)

            # weights stream through a rotating pool
            wpool = ctx.enter_context(tc.tile_pool(name="wpool", bufs=5))

            def load_w(name, kt=FT):
                wt = wpool.tile([128, kt, H], bf, name=f"{name}_sb", tag="wt")
                nc.sync.dma_start(
                    out=wt, in_=D[name].rearrange("(kt p) n -> p kt n", p=128))
                return wt

            BIAS = {}
            for l in range(L):
                for nm in ("bq", "bk", "b1"):  # per-partition, feature-major
                    if flags[f"{nm}{l}"]:
                        BIAS[f"{nm}{l}"] = consts.tile([128, FT], f32, name=f"{nm}{l}_sb")
                        nc.sync.dma_start(
                            out=BIAS[f"{nm}{l}"],
                            in_=D[f"{nm}{l}"].rearrange("(kt p) -> p kt", p=128))
                for nm in ("bv", "bfc", "b2"):  # broadcast, token-major
                    if flags[f"{nm}{l}"]:
                        BIAS[f"{nm}{l}"] = consts.tile([128, H], f32, name=f"{nm}{l}_sb")
                        nc.sync.dma_start(
                            out=BIAS[f"{nm}{l}"], in_=bcast_ap(D[f"{nm}{l}"], H))
                for nm in ("ln1", "ln2"):
                    if flags[f"{nm}{l}"]:
                        for gb in ("g", "b"):
                            BIAS[f"{nm}{gb}{l}"] = consts.tile([128, H], f32, name=f"{nm}{gb}{l}_sb")
                            nc.sync.dma_start(
                                out=BIAS[f"{nm}{gb}{l}"],
                                in_=bcast_ap(D[f"{nm}{gb}{l}"], H))

            # ---- persistent activations ----
            x_bf = acts.tile([128, NT, H], bf)       # token-major LN output
            xT = acts.tile([128, FT, H], bf)         # feature-major, 32/96 pad
            qT_f = [acts.tile([128, T_LOC], bf, name=f"qT{i}")
                    for i in range(FT)]
            kTp_f = [acts.tile([128, KPAD], bf, name=f"kTp{i}")
                     for i in range(FT)]
            V_ev = acts.tile([128, FT, H], bf)       # window m: [128m-32,128m+96)
            V_od = acts.tile([128, NT, H], bf)       # window m: [128m+32,128m+160)
            ctxTs = [acts.tile([128, FT, 128], bf, name=f"ctxT{i}")
                     for i in range(NT)]
            H1T = acts.tile([128, FT, T_LOC], bf)
            # odd heads' q/k features repacked to partitions 0:64 so score
            # matmuls never touch the row-64 PE quadrant (hw erratum: long
            # back-to-back K=64 runs at row offset 64 wedge the exec unit)
            kOdd_f = [acts.tile([64, KPAD], bf, name=f"kOdd{i}")
                      for i in range(FT)]
            qOdd_f = [acts.tile([64, T_LOC], bf, name=f"qOdd{i}")
                      for i in range(FT)]

            # one-time pad init (interiors are fully rewritten each layer)
            nc.vector.memset(xT[:, :, 0:32], 0.0)
            nc.vector.memset(xT[:, :, 32 + T_LOC:H], 0.0)
            for i in range(FT):
                nc.vector.memset(kTp_f[i][:, 0:32], 0.0)
                nc.vector.memset(kTp_f[i][:, 32 + T_LOC:KPAD], 0.0)

            def transpose_set(t, dst_f32=None):
                """PE-transpose x_bf tile t into xT[:, :, 32+128t:+128]."""
                trp = pstr.tile([128, FT, 128], bf, tag="trp")
                for ft in range(FT):
                    nc.tensor.transpose(
                        trp[:, ft, :],
                        x_bf[:, t, ft * 128:(ft + 1) * 128], ident_sb)
                nc.vector.tensor_copy(
                    xT[:, :, 32 + t * 128:32 + (t + 1) * 128], trp)

            def layernorm_act(F, l, ln, dst):
                """LN over F [128, H] f32 -> dst (token-major). Stats on DVE
                (bn_stats), normalize as one Act Identity with [P,1] APs."""
                stats = lpool.tile([128, 2, 6], f32, tag="stats")
                nc.vector.bn_stats(stats[:, 0, :], F[:, 0:384])
                nc.vector.bn_stats(stats[:, 1, :], F[:, 384:768])
                mv = lpool.tile([128, 2], f32, tag="mv")
                nc.vector.bn_aggr(mv, stats)
                sd = lpool.tile([128, 1], f32, tag="sd")
                nc.scalar.activation(sd, mv[:, 1:2], AF.Sqrt,
                                     bias=eps_sb[:, 0:1])
                rstd = lpool.tile([128, 1], f32, tag="rstd")
                nc.vector.reciprocal(rstd, sd)
                negm = lpool.tile([128, 1], f32, tag="negm")
                nc.vector.scalar_tensor_tensor(
                    out=negm, in0=mv[:, 0:1], scalar=-1.0, in1=rstd,
                    op0=ALU.mult, op1=ALU.mult)
                nc.scalar.activation(dst, F, AF.Identity, bias=negm[:, 0:1],
                                     scale=rstd[:, 0:1])
                if flags[f"{ln}{l}"]:
                    nc.vector.tensor_tensor(
                        out=dst, in0=dst, in1=BIAS[f"{ln}g{l}"], op=ALU.mult)
                    nc.vector.tensor_tensor(
                        out=dst, in0=dst, in1=BIAS[f"{ln}b{l}"], op=ALU.add)

            # ---- embedding ----
            wemb_a = consts.tile([128, 2, H], bf, name="wemb_a")
            wemb_b = consts.tile([128, 2, H], bf, name="wemb_b")
            nc.sync.dma_start(
                out=srcT_a,
                in_=D["srcT"][0:256, :].rearrange("(kt p) n -> p kt n", p=128))
            nc.sync.dma_start(
                out=wemb_a,
                in_=D["wembT"][0:256, :].rearrange("(kt p) n -> p kt n", p=128))
            nc.sync.dma_start(
                out=srcT_b,
                in_=D["srcT"][256:512, :].rearrange("(kt p) n -> p kt n", p=128))
            nc.sync.dma_start(
                out=wemb_b,
                in_=D["wembT"][256:512, :].rearrange("(kt p) n -> p kt n", p=128))
            pos_sb = consts.tile([128, NT, H], bf)
            nc.sync.dma_start(
                out=pos_sb, in_=D["pos_emb"].rearrange("(t p) n -> p t n", p=128))
            nc.sync.dma_start(out=ident_sb, in_=D["ident"][:, :])
            masks_sb = consts.tile([128, NB, 2, 384], bf)
            epools = [(psg, "gemm"), (pss, "S"), (psc, "cd")]
            ei = 0
            for t in range(NT):
                for c0 in (0, 384):
                    pl, tg = epools[ei % 3]; ei += 1
                    ps = pl.tile([128, 384], f32, tag=tg)
                    for kt in range(KTE):
                        sb = (srcT_a, srcT_b)[kt // 2]
                        wb = (wemb_a, wemb_b)[kt // 2]
                        nc.tensor.matmul(
                            ps, sb[:, kt % 2, t * 128:(t + 1) * 128],
                            wb[:, kt % 2, c0:c0 + 384],
                            start=(kt == 0), stop=(kt == KTE - 1))
                    nc.vector.tensor_add(
                        x_bf[:, t, c0:c0 + 384], ps, pos_sb[:, t, c0:c0 + 384])
                transpose_set(t)

            def dump_out():
                for t in range(NT):
                    xo = fpool.tile([128, H], f32, tag="xout")
                    nc.vector.tensor_copy(xo, x_bf[:, t, :])
                    lo = max(128 * t, HALO) - 128 * t
                    hi = min(128 * t + 128, HALO + CHUNK) - 128 * t
                    nc.sync.dma_start(
                        out=out_d[128 * t + lo - HALO:128 * t + hi - HALO, :],
                        in_=xo[lo:hi, :])

            # ---- layers ----
            for l in range(L):
                if LIMIT <= 10 * l + 1:
                    break
                # receptive-field shrink: layer 1 only needs queries on
                # [0,576) and keys/values on [0,608) to produce out[64:576]
                QCH = ((0, 320), (320, 320)) if l == 0 else ((0, 320), (320, 256))
                KCH = ((0, 320), (320, 320)) if l == 0 else ((0, 320), (320, 288))
                W1CH = ((0, 320), (320, 288)) if l == 0 else ((64, 320), (384, 192))
                NB_L = NB if l == 0 else 9
                VEV_L = FT if l == 0 else 5
                VOD_L = NT if l == 0 else 4
                # q/k GEMMs (feature-major outputs); copies on Act
                wq = load_w(f"wqT{l}") if l == 0 else wqn
                wk = load_w(f"wkT{l}") if l == 0 else wkn
                if l == 0:
                    nc.sync.dma_start(out=masks_sb, in_=D["masks"][:, :, :, :])
                qkpools = [(psg, "gemm"), (pss, "S"), (psc, "cd")]
                qki = 0
                for ft in range(FT):
                    for ci in (0, 1):
                        qc0, qn = QCH[ci]
                        pl, tg = qkpools[qki % 3]; qki += 1
                        psq = pl.tile([128, 384], f32, tag=tg)
                        for kt in range(FT):
                            nc.tensor.matmul(
                                psq[:, 0:qn], wq[:, kt, ft * 128:(ft + 1) * 128],
                                xT[:, kt, 32 + qc0:32 + qc0 + qn],
                                start=(kt == 0), stop=(kt == FT - 1))
                        if flags[f"bq{l}"]:
                            nc.scalar.activation(
                                qT_f[ft][:, qc0:qc0 + qn], psq[:, 0:qn],
                                AF.Identity, bias=BIAS[f"bq{l}"][:, ft:ft + 1])
                        else:
                            nc.scalar.activation(
                                qT_f[ft][:, qc0:qc0 + qn], psq[:, 0:qn], AF.Copy)
                        kc0, kn = KCH[ci]
                        pl, tg = qkpools[qki % 3]; qki += 1
                        psk = pl.tile([128, 384], f32, tag=tg)
                        for kt in range(FT):
                            nc.tensor.matmul(
                                psk[:, 0:kn], wk[:, kt, ft * 128:(ft + 1) * 128],
                                xT[:, kt, 32 + kc0:32 + kc0 + kn],
                                start=(kt == 0), stop=(kt == FT - 1))
                        if flags[f"bk{l}"]:
                            nc.vector.tensor_scalar_add(
                                kTp_f[ft][:, 32 + kc0:32 + kc0 + kn], psk[:, 0:kn],
                                BIAS[f"bk{l}"][:, ft:ft + 1])
                        else:
                            nc.vector.tensor_copy(
                                kTp_f[ft][:, 32 + kc0:32 + kc0 + kn], psk[:, 0:kn])
                        if ci == 1:
                            nc.gpsimd.dma_start(out=qOdd_f[ft],
                                                in_=qT_f[ft][64:128, :])
                            nc.gpsimd.dma_start(out=kOdd_f[ft],
                                                in_=kTp_f[ft][64:128, :])

                if LIMIT <= 10 * l + 2:
                    break

                # V GEMM: even shifted windows (padded xT -> uniform M=128)
                wv = load_w(f"wvT{l}")
                wfc = load_w(f"wfcT{l}")
                vi = 0

                def emit_vev(m):
                    nonlocal vi
                    for c0 in (0, 384):
                        pl, tg = qkpools[vi % 3]; vi += 1
                        psv = pl.tile([128, 384], f32, tag=tg)
                        for kt in range(FT):
                            nc.tensor.matmul(
                                psv, xT[:, kt, 128 * m:128 * m + 128],
                                wv[:, kt, c0:c0 + 384],
                                start=(kt == 0), stop=(kt == FT - 1))
                        if flags[f"bv{l}"]:
                            nc.vector.tensor_add(
                                V_ev[:, m, c0:c0 + 384], psv,
                                BIAS[f"bv{l}"][:, c0:c0 + 384])
                        else:
                            nc.vector.tensor_copy(V_ev[:, m, c0:c0 + 384], psv)

                def emit_vod(m):
                    # V_od[m] = tokens [128m+32, 128m+160): SBUF->SBUF shift
                    nc.gpsimd.dma_start(out=V_od[0:64, m, :],
                                        in_=V_ev[64:128, m, :])
                    nc.gpsimd.dma_start(out=V_od[64:128, m, :],
                                        in_=V_ev[0:64, m + 1, :])

                emit_vev(0)
                emit_vev(1)
                emit_vod(0)

                if LIMIT <= 10 * l + 3:
                    break
                # ---- attention: 20 units (block b, head-half g), skewed ----
                w1 = load_w(f"w1T{l}")
                unit_P = {}

                def attn_front(u):
                    b, g = divmod(u, 2)
                    Sp = pss.tile([128, 384], f32, tag="S")
                    for j in range(6):
                        h = 6 * g + j
                        hp, hs = divmod(h, 2)
                        kt_ = kTp_f[hp] if hs == 0 else kOdd_f[hp]
                        qt_ = qT_f[hp] if hs == 0 else qOdd_f[hp]
                        nc.tensor.matmul(
                            Sp[:, 64 * j:64 * j + 64],
                            kt_[0:64, 64 * b:64 * b + 128],
                            qt_[0:64, 64 * b:64 * b + 64],
                            start=True, stop=True)
                    Pt = ppool.tile([128, 384], bf, tag="P")
                    nc.scalar.activation(Pt, Sp, AF.Exp)
                    nc.gpsimd.tensor_tensor(
                        out=Pt, in0=Pt, in1=masks_sb[:, b, g, :], op=ALU.mult)
                    unit_P[u] = Pt

                def attn_back(u):
                    if ATTN_SUB < 2:
                        unit_P.pop(u)
                        return
                    b, g = divmod(u, 2)
                    Pt = unit_P.pop(u)
                    cd = psc.tile([128, 384], f32, tag="cd")
                    Vt = V_ev if b % 2 == 0 else V_od
                    m = b // 2
                    for j in range(3):
                        ha = 6 * g + 2 * j
                        ca, cb = 128 * j, 128 * j + 64
                        if ATTN_SUB >= 3:
                            nc.tensor.matmul(
                                cd[0:64, 64 * j:64 * j + 64],
                                Vt[:, m, 64 * ha:64 * ha + 64],
                                Pt[:, ca:ca + 64], start=True, stop=True)
                            nc.tensor.matmul(
                                cd[64:128, 64 * j:64 * j + 64],
                                Vt[:, m, 64 * ha + 64:64 * ha + 128],
                                Pt[:, cb:cb + 64], start=True, stop=True)
                        nc.tensor.matmul(
                            cd[0:64, 192 + 64 * j:256 + 64 * j],
                            ones64, Pt[:, ca:ca + 64], start=True, stop=True)
                        nc.tensor.matmul(
                            cd[64:128, 192 + 64 * j:256 + 64 * j],
                            ones64, Pt[:, cb:cb + 64], start=True, stop=True)
                    if ATTN_SUB < 4:
                        return
                    rb = rpool.tile([128, 192], f32, tag="rb")
                    _act_reciprocal(nc, mybir, rb, cd[:, 192:384])
                    if ATTN_SUB < 5 or ATTN_SUB == 6:
                        return
                    hb = 64 * (b % 2)
                    nc.vector.tensor_tensor(
                        out=ctxTs[b // 2][:, 3 * g:3 * g + 3, hb:hb + 64],
                        in0=cd[:, 0:192].rearrange("p (j q) -> p j q", j=3),
                        in1=rb.rearrange("p (j q) -> p j q", j=3),
                        op=ALU.mult)

                w2 = load_w(f"w2T{l}")
                if l < L - 1:
                    wqn = load_w(f"wqT{l + 1}")
                    wkn = load_w(f"wkT{l + 1}")

                def emit_fc(t):
                    F = fpool.tile([128, H], f32, tag="F")
                    for c0 in (0, 384):
                        ps = psg.tile([128, 384], f32, tag="gemm")
                        for kt in range(FT):
                            nc.tensor.matmul(
                                ps, ctxTs[t][:, kt, :],
                                wfc[:, kt, c0:c0 + 384],
                                start=(kt == 0), stop=(kt == FT - 1))
                        nc.vector.scalar_tensor_tensor(
                            out=F[:, c0:c0 + 384], in0=ps, scalar=1.0,
                            in1=x_bf[:, t, c0:c0 + 384],
                            op0=ALU.mult, op1=ALU.add)
                        if flags[f"bfc{l}"]:
                            nc.vector.tensor_add(
                                F[:, c0:c0 + 384], F[:, c0:c0 + 384],
                                BIAS[f"bfc{l}"][:, c0:c0 + 384])
                    layernorm_act(F, l, "ln1", x_bf[:, t, :])

                # two-unit skew between front (scores/exp/mask) and back
                # (den/ctx/recip/mult) hides the cross-engine chain latency;
                # fc(t) slots in once its two blocks' (4 units') mults are in
                for u in range(2 * NB_L + 3):
                    if u % 4 == 0:
                        mneed = u // 4 + 2
                        if mneed < VEV_L:
                            emit_vev(mneed)
                        if mneed - 1 < VOD_L:
                            emit_vod(mneed - 1)
                    fc_t = next((t_ for t_ in range(NT)
                                 if u == min(4 * t_ + 6, 2 * NB_L + 2)), None)
                    if fc_t is not None and u >= 3:
                        attn_back(u - 3)
                        if u < 2 * NB_L:
                            attn_front(u)
                    else:
                        if u < 2 * NB_L:
                            attn_front(u)
                        if u >= 3:
                            attn_back(u - 3)
                    if fc_t is not None:
                        if fc_t >= 1:
                            transpose_set(fc_t - 1)     # x1T, LN1 long done
                        emit_fc(fc_t)
                transpose_set(NT - 1)

                if LIMIT <= 10 * l + 5:
                    break
                # ---- FFN ----
                w1pools = [(psg, "gemm"), (pss, "S"), (psc, "cd")]

                def emit_w1(i2, ch, ft):
                    c0, cn = ch
                    pl, tg = w1pools[i2 % 3]
                    ps = pl.tile([128, 384], f32, tag=tg)
                    for kt in range(FT):
                        nc.tensor.matmul(
                            ps[:, 0:cn], w1[:, kt, ft * 128:(ft + 1) * 128],
                            xT[:, kt, 32 + c0:32 + c0 + cn],
                            start=(kt == 0), stop=(kt == FT - 1))
                    if flags[f"b1{l}"]:
                        nc.scalar.activation(
                            H1T[:, ft, c0:c0 + cn], ps[:, 0:cn], AF.Relu,
                            bias=BIAS[f"b1{l}"][:, ft:ft + 1])
                    elif i2 % 2 == 0:
                        nc.vector.tensor_scalar_max(
                            H1T[:, ft, c0:c0 + cn], ps[:, 0:cn], 0.0)
                    else:
                        nc.scalar.activation(
                            H1T[:, ft, c0:c0 + cn], ps[:, 0:cn], AF.Relu)

                for i2 in range(FT):
                    emit_w1(i2, W1CH[0], i2)

                if LIMIT <= 10 * l + 6:
                    break
                # W2 tiles inside W1's first token chunk interleave with the
                # second W1 chunk, hiding the relu turnaround
                c0a, cna = W1CH[0]
                early_w2 = [t for t in range(NT)
                            if c0a <= 128 * t and 128 * (t + 1) <= c0a + cna]
                w2_chunk2_at = (max(early_w2) + 1) if early_w2 else 0
                w2i = 0
                for t in range(NT):
                    if t == w2_chunk2_at:
                        for i2 in range(FT):
                            emit_w1(FT + i2, W1CH[1], i2)
                    if t >= 1 and l < L - 1:
                        transpose_set(t - 1)            # x2T, LN2 long done
                    F2 = fpool.tile([128, H], f32, tag="F")
                    for c0 in (0, 384):
                        pl, tg = w1pools[w2i % 3]; w2i += 1
                        ps = pl.tile([128, 384], f32, tag=tg)
                        for kt in range(FT):
                            nc.tensor.matmul(
                                ps, H1T[:, kt, 128 * t:128 * t + 128],
                                w2[:, kt, c0:c0 + 384],
                                start=(kt == 0), stop=(kt == FT - 1))
                        nc.vector.scalar_tensor_tensor(
                            out=F2[:, c0:c0 + 384], in0=ps, scalar=1.0,
                            in1=x_bf[:, t, c0:c0 + 384],
                            op0=ALU.mult, op1=ALU.add)
                        if flags[f"b2{l}"]:
                            nc.vector.tensor_add(
                                F2[:, c0:c0 + 384], F2[:, c0:c0 + 384],
                                BIAS[f"b2{l}"][:, c0:c0 + 384])
                    if l < L - 1:  # noqa
                        layernorm_act(F2, l, "ln2", x_bf[:, t, :])
                        if t == NT - 1:
                            transpose_set(t)
                    else:
                        xo = fpool.tile([128, H], f32, tag="xout")
                        layernorm_act(F2, l, "ln2", xo)
                        lo = max(128 * t, HALO) - 128 * t
                        hi = min(128 * t + 128, HALO + CHUNK) - 128 * t
                        nc.sync.dma_start(
                            out=out_d[128 * t + lo - HALO:128 * t + hi - HALO, :],
                            in_=xo[lo:hi, :])

            if LIMIT < 99:
                dump_out()

    _legalize_waits(nc)
    return nc, names


def run_on_device(shared, per_core, flags, trace=False):
    from concourse.bass_utils import run_bass_kernel_spmd

    nc, names = build_program(flags)
    in_maps = []
    for c in range(NCORES):
        m = {}
        for n in names:
            src = per_core[c] if n in per_core[c] else shared
            m[n] = np.ascontiguousarray(src[n])
        in_maps.append(m)
    res = run_bass_kernel_spmd(nc, in_maps, core_ids=list(range(NCORES)),
                               trace=trace)
    return [r["out"] for r in res.results], res


def kernel(**inputs):
    shared, per_core, flags = host_prep(inputs)
    core_outs, _ = run_on_device(shared, per_core, flags)
    return assemble(core_outs)
